# revision 22
# baseline (speedup 1.0000x reference)
"""Trainium2 Bass kernel for nn_ASVT_9500467658791 (ragged segment attention).

Pipeline (per point-cloud segment, one segment per NeuronCore, 8 cores):
  q/k/v = feat @ {Wq,Wk,Wv}  (1x1 convs)
  per-segment unscaled-softmax attention  r = softmax(q k^T) v
  t = r @ Wt ; BatchNorm over the full batch (training stats, synced across
  cores via tiny AllGathers) ; out = feat + relu(bn(t))

Layout strategy: everything d-major ("transposed") on chip.  The host
pre-transposes feat so no on-device transposes of the input are needed;
q^T/k^T are produced 4x-replicated across partition bands so the dqk=32
score matmuls can be packed 4-wide into the PE array with tile_position
(emitted back-to-back per group so the strips overlap in the array).
The q/k/score path runs in float32r (TF32-like).  Scores are computed
keys-major ([key, query]); the exp'd attention matrix streams as the
MOVING operand against stationary v-halves, accumulating r^T [d, q]
directly (no transposes anywhere).  Softmax denominators come from
mask-column matmuls (one per key tile, batched back-to-back so the
all-ones stationary stays resident), inverted with the fast DVE
reciprocal on all 128 partitions of the (identical-row) accumulator.
BN statistics are reduced along the free axis of t^T and synced in
three partial AllGathers: the first two overlap the remaining chunks'
compute and re-align core skew, so only the last (~1/5 of the data,
issued after only ~6us of post-barrier compute) sits on the tail.
The final output stays d-major (residual added from the exact f32 bits
of featT) and the host transposes it back during unsharding.
"""

import os
from contextlib import ExitStack

import numpy as np
import ml_dtypes

import concourse.bass as bass
import concourse.bacc as bacc
import concourse.tile as tile
from concourse import mybir
from concourse import bass_utils
from concourse.masks import make_identity

f32 = mybir.dt.float32
f32r = mybir.dt.float32r
bf16 = mybir.dt.bfloat16
AF = mybir.ActivationFunctionType
ALU = mybir.AluOpType
AX = mybir.AxisListType

NCORES = 8
D = 256
N_TOT = 16384
EPS = 1e-5
LP_MIN = 2176     # default segment pad (17 tiles); raised dynamically if needed

LAST_RESULT = None  # BassKernelResults of the most recent run (for test harness)
_NC_CACHE = {}


def _chunks(LP):
    out, c0 = [], 0
    while c0 < LP:
        out.append((c0, min(512, LP - c0)))
        c0 += 512
    return out


def build_nc(LP):
    NT = LP // 128
    chunks = _chunks(LP)
    nch = len(chunks)

    nc = bacc.Bacc("TRN2", target_bir_lowering=False, debug=False,
                   enable_asserts=True, num_devices=NCORES)

    featT_d = nc.dram_tensor("featT", [D, LP], f32r, kind="ExternalInput")
    maskf_d = nc.dram_tensor("maskf", [LP, 1], f32, kind="ExternalInput")
    maskr_d = nc.dram_tensor("maskr", [1, LP], f32, kind="ExternalInput")
    wqr_d = nc.dram_tensor("wqr", [D, 128], f32r, kind="ExternalInput")
    wkr_d = nc.dram_tensor("wkr", [D, 128], f32r, kind="ExternalInput")
    wv_d = nc.dram_tensor("wv", [D, D], f32r, kind="ExternalInput")
    wt_d = nc.dram_tensor("wt", [D, D], bf16, kind="ExternalInput")
    gamt_d = nc.dram_tensor("gamt", [D, 1], f32, kind="ExternalInput")
    bett_d = nc.dram_tensor("bett", [D, 1], f32, kind="ExternalInput")
    sel_d = nc.dram_tensor("sel", [4 * NCORES, 4], f32, kind="ExternalInput")
    out_d = nc.dram_tensor("out", [D, LP], f32, kind="ExternalOutput")

    # three partial-stats collectives + one warm-up
    cc_in = [nc.dram_tensor(f"cc_in{j}", [4, 128], f32, kind="Internal")
             for j in range(3)]
    cc_out = [nc.dram_tensor(f"cc_out{j}", [4 * NCORES, 128], f32,
                             kind="Internal", addr_space="Shared")
              for j in range(3)]
    ccw_in = nc.dram_tensor("ccw_in", [1, 128], f32, kind="Internal")
    ccw_out = nc.dram_tensor("ccw_out", [NCORES, 128], f32, kind="Internal",
                             addr_space="Shared")

    with tile.TileContext(nc) as tc, ExitStack() as ctx:
        const = ctx.enter_context(tc.tile_pool(name="const", bufs=1))
        big = ctx.enter_context(tc.tile_pool(name="big", bufs=1))
        vpool = ctx.enter_context(tc.tile_pool(name="vpool", bufs=1))
        epool = ctx.enter_context(tc.tile_pool(name="epool", bufs=2))
        work = ctx.enter_context(tc.tile_pool(name="work", bufs=3))
        small = ctx.enter_context(tc.tile_pool(name="small", bufs=4))
        # PSUM: 8 banks = scores 4 + rT accum 2 + denom 1 + tT/misc 1
        psS = ctx.enter_context(tc.tile_pool(name="psS", bufs=4, space="PSUM"))
        psV = ctx.enter_context(tc.tile_pool(name="psV", bufs=1, space="PSUM"))
        psD = ctx.enter_context(tc.tile_pool(name="psD", bufs=1, space="PSUM"))
        psA = ctx.enter_context(tc.tile_pool(name="psA", bufs=1, space="PSUM"))

        # ---------- input DMAs first (start as early as possible) ----------
        featT_sb = [big.tile([128, LP], f32r, tag=f"featT{h}", name=f"featT{h}")
                    for h in range(2)]
        for (c0, cw) in chunks:
            for h in range(2):
                nc.sync.dma_start(out=featT_sb[h][:, c0:c0 + cw],
                                  in_=featT_d[128 * h:128 * (h + 1), c0:c0 + cw])

        maskf_sb = const.tile([128, NT], f32, tag="maskf")
        nc.sync.dma_start(out=maskf_sb,
                          in_=maskf_d.rearrange("(n p) one -> p (n one)", p=128))
        maskbc_sb = const.tile([128, LP], f32, tag="maskbc")
        _mr = maskr_d[0:1, :]
        nc.sync.dma_start(out=maskbc_sb, in_=bass.AP(
            tensor=_mr.tensor, offset=_mr.offset, ap=[[0, 128]] + list(_mr.ap[1:])))

        wqr_sb = [const.tile([128, 128], f32r, tag=f"wqr{h}", name=f"wqr{h}")
                  for h in range(2)]
        wkr_sb = [const.tile([128, 128], f32r, tag=f"wkr{h}", name=f"wkr{h}")
                  for h in range(2)]
        wv_sb = [const.tile([128, D], f32r, tag=f"wv{h}", name=f"wv{h}")
                 for h in range(2)]
        wt_sb = [const.tile([128, D], bf16, tag=f"wt{h}", name=f"wt{h}")
                 for h in range(2)]
        gamt_sb = [const.tile([128, 1], f32, tag=f"gam{h}", name=f"gam{h}")
                   for h in range(2)]
        bett_sb = [const.tile([128, 1], f32, tag=f"bet{h}", name=f"bet{h}")
                   for h in range(2)]
        for h in range(2):
            sl = slice(128 * h, 128 * (h + 1))
            nc.sync.dma_start(out=wqr_sb[h], in_=wqr_d[sl, :])
            nc.sync.dma_start(out=wkr_sb[h], in_=wkr_d[sl, :])
            nc.sync.dma_start(out=wv_sb[h], in_=wv_d[sl, :])
            nc.sync.dma_start(out=wt_sb[h], in_=wt_d[sl, :])
            nc.sync.dma_start(out=gamt_sb[h], in_=gamt_d[sl, :])
            nc.sync.dma_start(out=bett_sb[h], in_=bett_d[sl, :])
        sel_sb = const.tile([4 * NCORES, 4], f32, tag="sel")
        nc.sync.dma_start(out=sel_sb, in_=sel_d[:, :])

        maskb_sb = const.tile([128, NT], bf16, tag="maskb")
        nc.vector.tensor_copy(out=maskb_sb, in_=maskf_sb)
        ones128 = const.tile([128, 128], bf16, tag="ones128")
        nc.vector.memset(ones128, 1.0)
        msk128 = const.tile([128, 128], bf16, tag="msk128")
        _mb = maskb_sb[:, NT - 1:NT]
        nc.vector.tensor_copy(out=msk128, in_=bass.AP(
            tensor=_mb.tensor, offset=_mb.offset, ap=[list(_mb.ap[0]), [0, 128]]))
        ident = const.tile([128, 128], f32, tag="ident")
        make_identity(nc, ident)
        ident_b = const.tile([128, 128], bf16, tag="ident_b")
        nc.vector.tensor_copy(out=ident_b, in_=ident)

        # ---------- PE clock warm-up: junk matmuls during the input DMAs ----
        ps_w = psA.tile([128, 512], f32, tag="a")
        for i in range(180):
            nc.tensor.matmul(ps_w[:, 0:128], lhsT=ident_b, rhs=ident_b,
                             start=True, stop=True)
        warm_junk = const.tile([128, 1], f32, tag="warm_junk")
        nc.vector.tensor_copy(out=warm_junk, in_=ps_w[:, 0:1])

        # ---------- warm-up collective (runs on TOPSP during phase A) -------
        wz = const.tile([1, 128], f32, tag="wz")
        nc.vector.memset(wz, 0.0)
        nc.sync.dma_start(out=ccw_in[:, :], in_=wz)
        nc.gpsimd.collective_compute(
            "AllGather", ALU.bypass, replica_groups=[list(range(NCORES))],
            ins=[ccw_in[:, :]], outs=[ccw_out[:, :]])

        # ---------- phase A: projections ----------
        # qT_rep / kT_rep [128, LP]: each 32-row band holds the full
        # [32, LP] q^T / k^T (host replicated W 4x along columns).
        qT_sb = big.tile([128, LP], f32r, tag="qT")
        kT_sb = big.tile([128, LP], f32r, tag="kT")
        for (c0, cw) in chunks:
            csl = slice(c0, c0 + cw)
            for wrep, dst in ((wqr_sb, qT_sb), (wkr_sb, kT_sb)):
                ps = psS.tile([128, 512], f32, tag="s")
                nc.tensor.matmul(ps[:, :cw], lhsT=wrep[0], rhs=featT_sb[0][:, csl],
                                 start=True, stop=False)
                nc.tensor.matmul(ps[:, :cw], lhsT=wrep[1], rhs=featT_sb[1][:, csl],
                                 start=False, stop=True)
                nc.vector.tensor_copy(out=dst[:, csl], in_=ps[:, :cw])

        v_sb = []
        for i in range(NT):
            isl = slice(128 * i, 128 * (i + 1))
            ps = psS.tile([128, 512], f32, tag="s", name=f"psv{i}")
            nc.tensor.matmul(ps[:, 0:D], lhsT=featT_sb[0][:, isl], rhs=wv_sb[0],
                             start=True, stop=False)
            nc.tensor.matmul(ps[:, 0:D], lhsT=featT_sb[1][:, isl], rhs=wv_sb[1],
                             start=False, stop=True)
            vt = vpool.tile([128, D], bf16, tag=f"v{i}", name=f"v{i}")
            nc.vector.tensor_copy(out=vt, in_=ps[:, 0:D])
            v_sb.append(vt)

        rT_sb = [big.tile([128, LP], bf16, tag=f"rT{h}", name=f"rT{h}")
                 for h in range(2)]
        tT_sb = [big.tile([128, LP], f32, tag=f"tT{h}", name=f"tT{h}")
                 for h in range(2)]
        sums_t = [const.tile([128, nch], f32, tag=f"st{h}", name=f"st{h}")
                  for h in range(2)]
        sums_q = [const.tile([128, nch], f32, tag=f"sq{h}", name=f"sq{h}")
                  for h in range(2)]

        def emit_stats_ag(j, lo, hi):
            """Pack partial sums over chunks [lo, hi) and AllGather them."""
            stf = const.tile([128, 4], f32, tag=f"stf{j}", name=f"stf{j}")
            for h in range(2):
                nc.vector.reduce_sum(out=stf[:, h:h + 1],
                                     in_=sums_t[h][:, lo:hi], axis=AX.X)
                nc.vector.reduce_sum(out=stf[:, 2 + h:3 + h],
                                     in_=sums_q[h][:, lo:hi], axis=AX.X)
            ps_st = psD.tile([4, 128], f32, tag="d", name=f"ps_st{j}")
            nc.tensor.transpose(ps_st, stf, ident)
            stp = const.tile([4, 128], f32, tag=f"stp{j}", name=f"stp{j}")
            nc.vector.tensor_copy(out=stp, in_=ps_st)
            nc.sync.dma_start(out=cc_in[j][:, :], in_=stp)
            nc.gpsimd.collective_compute(
                "AllGather", ALU.bypass, replica_groups=[list(range(NCORES))],
                ins=[cc_in[j][:, :]], outs=[cc_out[j][:, :]])

        # ---------- phases B-D: attention + r^T + t^T, chunked over queries --
        for ci, (c0, cw) in enumerate(chunks):
            csl = slice(c0, c0 + cw)

            ps_rt = [psV.tile([128, 512], f32, tag=f"v{h}", name=f"psv{h}")
                     for h in range(2)]
            ps_d = psD.tile([128, 512], f32, tag="d")

            # (1) scores, 4-way row-packed and back-to-back per group,
            #     exp'd per key tile into kt-pair SBUF tiles
            exp_of = {}
            score_ps = {}
            for g0 in range(0, NT, 4):
                grp = list(range(g0, min(g0 + 4, NT)))
                for i, kt in enumerate(grp):
                    pss = psS.tile([128, 512], f32, tag="s", name=f"pss{kt % 4}")
                    ksl = slice(128 * kt, 128 * (kt + 1))
                    bsl = slice(32 * i, 32 * (i + 1))
                    nc.tensor.matmul(pss[:, :cw], lhsT=kT_sb[bsl, ksl],
                                     rhs=qT_sb[bsl, csl],
                                     start=True, stop=True,
                                     tile_position=(32 * i, 0))
                    score_ps[kt] = pss
                for kt in grp:
                    pi, sub = kt // 2, kt % 2
                    if sub == 0:
                        et = epool.tile([128, 1024], bf16, tag=f"e{pi}",
                                        name=f"e{pi}")
                    else:
                        et = exp_of[kt - 1][0]
                    nc.scalar.activation(out=et[:, sub * cw:sub * cw + cw],
                                         in_=score_ps[kt][:, :cw], func=AF.Exp)
                    exp_of[kt] = (et, sub)

            # (2) attention @ v: r^T accumulation, v-halves stationary
            for kt in range(NT):
                et, sub = exp_of[kt]
                esl = slice(sub * cw, sub * cw + cw)
                for h in range(2):
                    nc.tensor.matmul(ps_rt[h][:, :cw],
                                     lhsT=v_sb[kt][:, 128 * h:128 * (h + 1)],
                                     rhs=et[:, esl],
                                     start=(kt == 0), stop=(kt == NT - 1))

            # (3) denominators: mask-column matmuls, ones-stationary batched
            for kt in range(NT):
                et, sub = exp_of[kt]
                nc.tensor.matmul(ps_d[:, :cw],
                                 lhsT=(msk128 if kt == NT - 1 else ones128),
                                 rhs=et[:, sub * cw:sub * cw + cw],
                                 start=(kt == 0), stop=(kt == NT - 1))

            # (4) masked reciprocal (all rows of ps_d are identical)
            dnf = work.tile([128, 512], f32, tag="dnf")
            nc.vector.tensor_scalar_max(out=dnf[:, :cw], in0=ps_d[:, :cw],
                                        scalar1=1e-30)
            rec = work.tile([128, 512], f32, tag="recd")
            nc.vector.reciprocal_approx_fast(out=rec[:, :cw], in_=dnf[:, :cw])
            nc.vector.tensor_mul(out=rec[:, :cw], in0=rec[:, :cw],
                                 in1=maskbc_sb[:, csl])
            for h in range(2):
                nc.vector.tensor_mul(out=rT_sb[h][:, csl], in0=ps_rt[h][:, :cw],
                                     in1=rec[:, :cw])

            # (5) tT = Wt^T @ rT + BN partial stats
            for h in range(2):
                hsl = slice(128 * h, 128 * (h + 1))
                ps_t = psA.tile([128, 512], f32, tag="a")
                nc.tensor.matmul(ps_t[:, :cw], lhsT=wt_sb[0][:, hsl],
                                 rhs=rT_sb[0][:, csl], start=True, stop=False)
                nc.tensor.matmul(ps_t[:, :cw], lhsT=wt_sb[1][:, hsl],
                                 rhs=rT_sb[1][:, csl], start=False, stop=True)
                nc.scalar.activation(out=tT_sb[h][:, csl], in_=ps_t[:, :cw],
                                     func=AF.Copy,
                                     accum_out=sums_t[h][:, ci:ci + 1])
                sq = work.tile([128, 512], f32, tag="sq")
                nc.vector.tensor_mul(out=sq[:, :cw], in0=tT_sb[h][:, csl],
                                     in1=tT_sb[h][:, csl])
                nc.vector.reduce_sum(out=sums_q[h][:, ci:ci + 1], in_=sq[:, :cw],
                                     axis=AX.X)

            # partial-stats collectives: overlap remaining compute + resync
            if ci == nch - 3:
                emit_stats_ag(0, 0, nch - 2)
            elif ci == nch - 2:
                emit_stats_ag(1, nch - 2, nch - 1)

        # ---------- phase E: last partial + combine global BN stats ----------
        emit_stats_ag(2, nch - 1, nch)
        ag_sb = []
        for j in range(3):
            a = const.tile([4 * NCORES, 128], f32, tag=f"ag{j}", name=f"ag{j}")
            nc.sync.dma_start(out=a, in_=cc_out[j][:, :])
            ag_sb.append(a)
        ps_g = psA.tile([128, 512], f32, tag="a", name="ps_g")
        for j in range(3):
            nc.tensor.matmul(ps_g[:, 0:4], lhsT=ag_sb[j], rhs=sel_sb,
                             start=(j == 0), stop=(j == 2))
        statsT = const.tile([128, 4], f32, tag="statsT")
        nc.vector.tensor_copy(out=statsT, in_=ps_g[:, 0:4])

        scale_h, bias_h = [], []
        inv_n = 1.0 / float(N_TOT)
        for h in range(2):
            mu = small.tile([128, 1], f32, tag="mu")
            nc.vector.tensor_scalar_mul(out=mu, in0=statsT[:, h:h + 1], scalar1=inv_n)
            musq = small.tile([128, 1], f32, tag="musq")
            nc.vector.tensor_mul(out=musq, in0=mu, in1=mu)
            msq = small.tile([128, 1], f32, tag="msq")
            nc.vector.tensor_scalar(out=msq, in0=statsT[:, 2 + h:3 + h],
                                    scalar1=inv_n, scalar2=None, op0=ALU.mult)
            varp = small.tile([128, 1], f32, tag="varp")
            nc.vector.tensor_sub(out=varp, in0=msq, in1=musq)
            nc.vector.tensor_scalar_add(out=varp, in0=varp, scalar1=EPS)
            sd = small.tile([128, 1], f32, tag="sd")
            nc.scalar.activation(out=sd, in_=varp, func=AF.Sqrt)
            rsig = small.tile([128, 1], f32, tag="rsig")
            nc.vector.reciprocal(out=rsig, in_=sd)
            # one Newton step: rsig' = rsig * (1.5 - 0.5 * varp * rsig^2)
            t1 = small.tile([128, 1], f32, tag="t1")
            nc.vector.tensor_mul(out=t1, in0=rsig, in1=rsig)
            t2 = small.tile([128, 1], f32, tag="t2")
            nc.vector.tensor_mul(out=t2, in0=t1, in1=varp)
            nc.vector.tensor_scalar(out=t2, in0=t2, scalar1=-0.5, scalar2=1.5,
                                    op0=ALU.mult, op1=ALU.add)
            nc.vector.tensor_mul(out=rsig, in0=rsig, in1=t2)
            sc = small.tile([128, 1], f32, tag="sc")
            nc.vector.tensor_mul(out=sc, in0=rsig, in1=gamt_sb[h])
            bi = small.tile([128, 1], f32, tag="bi")
            nc.vector.tensor_mul(out=bi, in0=mu, in1=sc)
            nc.vector.tensor_sub(out=bi, in0=bett_sb[h], in1=bi)
            scale_h.append(sc)
            bias_h.append(bi)

        # ---------- phase F: BN apply + relu + residual (stays d-major) -----
        for h in range(2):
            relu_sb = big.tile([128, LP], f32, tag=f"relu{h}", name=f"relu{h}")
            nc.scalar.activation(out=relu_sb, in_=tT_sb[h],
                                 func=AF.Relu, bias=bias_h[h], scale=scale_h[h])
            o_sb = big.tile([128, LP], f32, tag=f"o{h}", name=f"o{h}")
            nc.vector.tensor_add(out=o_sb, in0=relu_sb,
                                 in1=featT_sb[h].bitcast(f32))
            nc.sync.dma_start(out=out_d[128 * h:128 * (h + 1), :], in_=o_sb)

    nc.compile()
    return nc


def _get_nc(LP):
    if LP not in _NC_CACHE:
        _NC_CACHE[LP] = build_nc(LP)
    return _NC_CACHE[LP]


def kernel(**inputs):
    global LAST_RESULT
    feat = np.asarray(inputs["feat"], dtype=np.float32)
    bids = np.asarray(inputs["bids"])
    Wq = np.asarray(inputs["Wq"], dtype=np.float32)
    Wk = np.asarray(inputs["Wk"], dtype=np.float32)
    Wv = np.asarray(inputs["Wv"], dtype=np.float32)
    Wt = np.asarray(inputs["Wt"], dtype=np.float32)
    gamma = np.asarray(inputs["gamma"], dtype=np.float32)
    beta = np.asarray(inputs["beta"], dtype=np.float32)

    n, d = feat.shape
    assert d == D
    starts = np.searchsorted(bids, np.arange(NCORES)).astype(np.int64)
    ends = np.append(starts[1:], n)
    lens = (ends - starts).astype(np.int64)
    maxlen = int(lens.max())
    LP = max(LP_MIN, ((maxlen + 127) // 128) * 128)
    nc = _get_nc(LP)

    wqr = np.ascontiguousarray(np.concatenate([Wq] * 4, axis=1))
    wkr = np.ascontiguousarray(np.concatenate([Wk] * 4, axis=1))
    wv = np.ascontiguousarray(Wv)
    wt = Wt.astype(ml_dtypes.bfloat16)
    gamt = gamma.reshape(D, 1).copy()
    bett = beta.reshape(D, 1).copy()
    sel = np.zeros((4 * NCORES, 4), dtype=np.float32)
    for p in range(4 * NCORES):
        sel[p, p % 4] = 1.0

    in_maps = []
    for c in range(NCORES):
        seg = feat[starts[c]:ends[c]]
        L = seg.shape[0]
        featT = np.zeros((D, LP), dtype=np.float32)
        featT[:, :L] = seg.T
        maskf = np.zeros((LP, 1), dtype=np.float32)
        maskf[:L] = 1.0
        in_maps.append({
            "featT": featT, "maskf": maskf,
            "maskr": np.ascontiguousarray(maskf.reshape(1, LP)),
            "wqr": wqr, "wkr": wkr, "wv": wv, "wt": wt,
            "gamt": gamt, "bett": bett, "sel": sel,
        })

    trace_cores = None
    if os.environ.get("BASS_TRACE"):
        trace_cores = list(range(NCORES))
    res = bass_utils.run_bass_kernel_spmd(
        nc, in_maps, core_ids=list(range(NCORES)), trace_cores=trace_cores)
    LAST_RESULT = res

    out = np.empty((n, D), dtype=np.float32)
    for c in range(NCORES):
        out[starts[c]:ends[c]] = res.results[c]["out"].T[:lens[c]]
    return out


# revision 23
# speedup vs baseline: 1.0617x; 1.0617x over previous
"""Trainium2 Bass kernel for nn_ASVT_9500467658791 (ragged segment attention).

Pipeline (per point-cloud segment, one segment per NeuronCore, 8 cores):
  q/k/v = feat @ {Wq,Wk,Wv}  (1x1 convs)
  per-segment unscaled-softmax attention  r = softmax(q k^T) v
  t = r @ Wt ; BatchNorm over the full batch (training stats, synced across
  cores via tiny AllGathers) ; out = feat + relu(bn(t))

Layout strategy: everything d-major ("transposed") on chip.  The host
pre-transposes feat so no on-device transposes of the input are needed;
q^T/k^T are produced 4x-replicated across partition bands so the dqk=32
score matmuls can be packed 4-wide into the PE array with tile_position
(emitted back-to-back per group so the strips overlap in the array).
The q/k/score path runs in float32r (TF32-like).  Scores are computed
keys-major ([key, query]); the exp'd attention matrix streams as the
MOVING operand against stationary v-halves, accumulating r^T [d, q]
directly (no transposes anywhere).  Softmax denominators come from
mask-column matmuls (one per key tile, batched back-to-back so the
all-ones stationary stays resident), inverted with the fast DVE
reciprocal on all 128 partitions of the (identical-row) accumulator.
BN statistics are reduced along the free axis of t^T and synced in
three partial AllGathers: the first two overlap the remaining chunks'
compute and re-align core skew, so only the last (~1/5 of the data,
issued after only ~6us of post-barrier compute) sits on the tail.
The final output stays d-major (residual added from the exact f32 bits
of featT) and the host transposes it back during unsharding.
"""

import os
from contextlib import ExitStack

import numpy as np
import ml_dtypes

import concourse.bass as bass
import concourse.bacc as bacc
import concourse.tile as tile
from concourse import mybir
from concourse import bass_utils
from concourse.masks import make_identity

f32 = mybir.dt.float32
f32r = mybir.dt.float32r
bf16 = mybir.dt.bfloat16
AF = mybir.ActivationFunctionType
ALU = mybir.AluOpType
AX = mybir.AxisListType

NCORES = 8
D = 256
N_TOT = 16384
EPS = 1e-5
LP_MIN = 2176     # default segment pad (17 tiles); raised dynamically if needed

LAST_RESULT = None  # BassKernelResults of the most recent run (for test harness)
_NC_CACHE = {}


def _chunks(LP):
    out, c0 = [], 0
    while c0 < LP:
        out.append((c0, min(512, LP - c0)))
        c0 += 512
    return out


def build_nc(LP):
    NT = LP // 128
    chunks = _chunks(LP)
    nch = len(chunks)

    nc = bacc.Bacc("TRN2", target_bir_lowering=False, debug=False,
                   enable_asserts=True, num_devices=NCORES)

    featT_d = nc.dram_tensor("featT", [D, LP], f32r, kind="ExternalInput")
    maskf_d = nc.dram_tensor("maskf", [LP, 1], f32, kind="ExternalInput")
    maskr_d = nc.dram_tensor("maskr", [1, LP], f32, kind="ExternalInput")
    wqr_d = nc.dram_tensor("wqr", [D, 128], f32r, kind="ExternalInput")
    wkr_d = nc.dram_tensor("wkr", [D, 128], f32r, kind="ExternalInput")
    wv_d = nc.dram_tensor("wv", [D, D], f32r, kind="ExternalInput")
    wt_d = nc.dram_tensor("wt", [D, D], bf16, kind="ExternalInput")
    gamt_d = nc.dram_tensor("gamt", [D, 1], f32, kind="ExternalInput")
    bett_d = nc.dram_tensor("bett", [D, 1], f32, kind="ExternalInput")
    sel_d = nc.dram_tensor("sel", [4 * NCORES, 4], f32, kind="ExternalInput")
    out_d = nc.dram_tensor("out", [D, LP], f32, kind="ExternalOutput")

    # three partial-stats collectives + one warm-up
    cc_in = [nc.dram_tensor(f"cc_in{j}", [4, 128], f32, kind="Internal")
             for j in range(3)]
    cc_out = [nc.dram_tensor(f"cc_out{j}", [4 * NCORES, 128], f32,
                             kind="Internal", addr_space="Shared")
              for j in range(3)]
    ccw_in = nc.dram_tensor("ccw_in", [1, 128], f32, kind="Internal")
    ccw_out = nc.dram_tensor("ccw_out", [NCORES, 128], f32, kind="Internal",
                             addr_space="Shared")

    with tile.TileContext(nc) as tc, ExitStack() as ctx:
        const = ctx.enter_context(tc.tile_pool(name="const", bufs=1))
        big = ctx.enter_context(tc.tile_pool(name="big", bufs=1))
        vpool = ctx.enter_context(tc.tile_pool(name="vpool", bufs=1))
        epool = ctx.enter_context(tc.tile_pool(name="epool", bufs=2))
        work = ctx.enter_context(tc.tile_pool(name="work", bufs=3))
        small = ctx.enter_context(tc.tile_pool(name="small", bufs=4))
        # PSUM: 8 banks = scores 4 + rT accum 2 + denom 1 + tT/misc 1
        psS = ctx.enter_context(tc.tile_pool(name="psS", bufs=4, space="PSUM"))
        psV = ctx.enter_context(tc.tile_pool(name="psV", bufs=1, space="PSUM"))
        psD = ctx.enter_context(tc.tile_pool(name="psD", bufs=1, space="PSUM"))
        psA = ctx.enter_context(tc.tile_pool(name="psA", bufs=1, space="PSUM"))

        # ---------- input DMAs first (start as early as possible) ----------
        wqr_sb = [const.tile([128, 128], f32r, tag=f"wqr{h}", name=f"wqr{h}")
                  for h in range(2)]
        wkr_sb = [const.tile([128, 128], f32r, tag=f"wkr{h}", name=f"wkr{h}")
                  for h in range(2)]
        for h in range(2):
            sl = slice(128 * h, 128 * (h + 1))
            nc.sync.dma_start(out=wqr_sb[h], in_=wqr_d[sl, :])
            nc.sync.dma_start(out=wkr_sb[h], in_=wkr_d[sl, :])
        featT_sb = [big.tile([128, LP], f32r, tag=f"featT{h}", name=f"featT{h}")
                    for h in range(2)]
        for (c0, cw) in chunks:
            for h in range(2):
                nc.sync.dma_start(out=featT_sb[h][:, c0:c0 + cw],
                                  in_=featT_d[128 * h:128 * (h + 1), c0:c0 + cw])

        maskf_sb = const.tile([128, NT], f32, tag="maskf")
        nc.sync.dma_start(out=maskf_sb,
                          in_=maskf_d.rearrange("(n p) one -> p (n one)", p=128))
        maskbc_sb = const.tile([128, LP], f32, tag="maskbc")
        _mr = maskr_d[0:1, :]
        nc.sync.dma_start(out=maskbc_sb, in_=bass.AP(
            tensor=_mr.tensor, offset=_mr.offset, ap=[[0, 128]] + list(_mr.ap[1:])))

        wv_sb = [const.tile([128, D], f32r, tag=f"wv{h}", name=f"wv{h}")
                 for h in range(2)]
        wt_sb = [const.tile([128, D], bf16, tag=f"wt{h}", name=f"wt{h}")
                 for h in range(2)]
        gamt_sb = [const.tile([128, 1], f32, tag=f"gam{h}", name=f"gam{h}")
                   for h in range(2)]
        bett_sb = [const.tile([128, 1], f32, tag=f"bet{h}", name=f"bet{h}")
                   for h in range(2)]
        for h in range(2):
            sl = slice(128 * h, 128 * (h + 1))
            nc.sync.dma_start(out=wv_sb[h], in_=wv_d[sl, :])
            nc.sync.dma_start(out=wt_sb[h], in_=wt_d[sl, :])
            nc.sync.dma_start(out=gamt_sb[h], in_=gamt_d[sl, :])
            nc.sync.dma_start(out=bett_sb[h], in_=bett_d[sl, :])
        sel_sb = const.tile([4 * NCORES, 4], f32, tag="sel")
        nc.sync.dma_start(out=sel_sb, in_=sel_d[:, :])

        maskb_sb = const.tile([128, NT], bf16, tag="maskb")
        nc.vector.tensor_copy(out=maskb_sb, in_=maskf_sb)
        ones128 = const.tile([128, 128], bf16, tag="ones128")
        nc.vector.memset(ones128, 1.0)
        msk128 = const.tile([128, 128], bf16, tag="msk128")
        _mb = maskb_sb[:, NT - 1:NT]
        nc.vector.tensor_copy(out=msk128, in_=bass.AP(
            tensor=_mb.tensor, offset=_mb.offset, ap=[list(_mb.ap[0]), [0, 128]]))
        ident = const.tile([128, 128], f32, tag="ident")
        make_identity(nc, ident)
        ident_b = const.tile([128, 128], bf16, tag="ident_b")
        nc.vector.tensor_copy(out=ident_b, in_=ident)

        # ---------- PE clock warm-up: junk matmuls during the input DMAs ----
        ps_w = psA.tile([128, 512], f32, tag="a")
        for i in range(60):
            nc.tensor.matmul(ps_w[:, 0:128], lhsT=ident_b, rhs=ident_b,
                             start=True, stop=True)
        warm_junk = const.tile([128, 1], f32, tag="warm_junk")
        nc.vector.tensor_copy(out=warm_junk, in_=ps_w[:, 0:1])

        # ---------- warm-up collective (runs on TOPSP during phase A) -------
        wz = const.tile([1, 128], f32, tag="wz")
        nc.vector.memset(wz, 0.0)
        nc.sync.dma_start(out=ccw_in[:, :], in_=wz)
        nc.gpsimd.collective_compute(
            "AllGather", ALU.bypass, replica_groups=[list(range(NCORES))],
            ins=[ccw_in[:, :]], outs=[ccw_out[:, :]])

        # ---------- phase A: projections ----------
        # qT_rep / kT_rep [128, LP]: each 32-row band holds the full
        # [32, LP] q^T / k^T (host replicated W 4x along columns).
        qT_sb = big.tile([128, LP], f32r, tag="qT")
        kT_sb = big.tile([128, LP], f32r, tag="kT")
        for (c0, cw) in chunks:
            csl = slice(c0, c0 + cw)
            for wrep, dst in ((wqr_sb, qT_sb), (wkr_sb, kT_sb)):
                ps = psS.tile([128, 512], f32, tag="s")
                nc.tensor.matmul(ps[:, :cw], lhsT=wrep[0], rhs=featT_sb[0][:, csl],
                                 start=True, stop=False)
                nc.tensor.matmul(ps[:, :cw], lhsT=wrep[1], rhs=featT_sb[1][:, csl],
                                 start=False, stop=True)
                nc.vector.tensor_copy(out=dst[:, csl], in_=ps[:, :cw])

        v_sb = []
        for i in range(NT):
            isl = slice(128 * i, 128 * (i + 1))
            ps = psS.tile([128, 512], f32, tag="s", name=f"psv{i}")
            nc.tensor.matmul(ps[:, 0:D], lhsT=featT_sb[0][:, isl], rhs=wv_sb[0],
                             start=True, stop=False)
            nc.tensor.matmul(ps[:, 0:D], lhsT=featT_sb[1][:, isl], rhs=wv_sb[1],
                             start=False, stop=True)
            vt = vpool.tile([128, D], bf16, tag=f"v{i}", name=f"v{i}")
            nc.vector.tensor_copy(out=vt, in_=ps[:, 0:D])
            v_sb.append(vt)

        rT_sb = [big.tile([128, LP], bf16, tag=f"rT{h}", name=f"rT{h}")
                 for h in range(2)]
        tT_sb = [big.tile([128, LP], f32, tag=f"tT{h}", name=f"tT{h}")
                 for h in range(2)]
        sums_t = [const.tile([128, nch], f32, tag=f"st{h}", name=f"st{h}")
                  for h in range(2)]
        sums_q = [const.tile([128, nch], f32, tag=f"sq{h}", name=f"sq{h}")
                  for h in range(2)]

        def emit_stats_ag(j, lo, hi):
            """Pack partial sums over chunks [lo, hi) and AllGather them."""
            stf = const.tile([128, 4], f32, tag=f"stf{j}", name=f"stf{j}")
            for h in range(2):
                nc.vector.reduce_sum(out=stf[:, h:h + 1],
                                     in_=sums_t[h][:, lo:hi], axis=AX.X)
                nc.vector.reduce_sum(out=stf[:, 2 + h:3 + h],
                                     in_=sums_q[h][:, lo:hi], axis=AX.X)
            ps_st = psD.tile([4, 128], f32, tag="d", name=f"ps_st{j}")
            nc.tensor.transpose(ps_st, stf, ident)
            stp = const.tile([4, 128], f32, tag=f"stp{j}", name=f"stp{j}")
            nc.vector.tensor_copy(out=stp, in_=ps_st)
            nc.sync.dma_start(out=cc_in[j][:, :], in_=stp)
            nc.gpsimd.collective_compute(
                "AllGather", ALU.bypass, replica_groups=[list(range(NCORES))],
                ins=[cc_in[j][:, :]], outs=[cc_out[j][:, :]])

        # ---------- phases B-D: attention + r^T + t^T, chunked over queries --
        for ci, (c0, cw) in enumerate(chunks):
            csl = slice(c0, c0 + cw)

            ps_rt = [psV.tile([128, 512], f32, tag=f"v{h}", name=f"psv{h}")
                     for h in range(2)]
            ps_d = psD.tile([128, 512], f32, tag="d")

            # (1) scores (4-band row packing) + exp + r^T accumulation,
            #     interleaved per kt-pair to keep the PE stream dense
            exp_of = {}
            pairs = [(kt, kt + 1 if kt + 1 < NT else None)
                     for kt in range(0, NT, 2)]
            for pi, (ka, kb) in enumerate(pairs):
                kts = [ka] + ([kb] if kb is not None else [])
                et = epool.tile([128, 1024], bf16, tag=f"e{pi}", name=f"e{pi}")
                for sub, kt in enumerate(kts):
                    i = kt % 4
                    pss = psS.tile([128, 512], f32, tag="s", name=f"pss{kt % 4}")
                    ksl = slice(128 * kt, 128 * (kt + 1))
                    bsl = slice(32 * i, 32 * (i + 1))
                    nc.tensor.matmul(pss[:, :cw], lhsT=kT_sb[bsl, ksl],
                                     rhs=qT_sb[bsl, csl],
                                     start=True, stop=True,
                                     tile_position=(32 * i, 0))
                    nc.scalar.activation(out=et[:, sub * cw:sub * cw + cw],
                                         in_=pss[:, :cw], func=AF.Exp)
                    exp_of[kt] = (et, sub)
                for sub, kt in enumerate(kts):
                    esl = slice(sub * cw, sub * cw + cw)
                    for h in range(2):
                        nc.tensor.matmul(ps_rt[h][:, :cw],
                                         lhsT=v_sb[kt][:, 128 * h:128 * (h + 1)],
                                         rhs=et[:, esl],
                                         start=(kt == 0), stop=(kt == NT - 1))

            # (3) denominators: mask-column matmuls, ones-stationary batched
            for kt in range(NT):
                et, sub = exp_of[kt]
                nc.tensor.matmul(ps_d[:, :cw],
                                 lhsT=(msk128 if kt == NT - 1 else ones128),
                                 rhs=et[:, sub * cw:sub * cw + cw],
                                 start=(kt == 0), stop=(kt == NT - 1))

            # (4) masked reciprocal (all rows of ps_d are identical)
            dnf = work.tile([128, 512], f32, tag="dnf")
            nc.vector.tensor_scalar_max(out=dnf[:, :cw], in0=ps_d[:, :cw],
                                        scalar1=1e-30)
            rec = work.tile([128, 512], f32, tag="recd")
            nc.vector.reciprocal_approx_fast(out=rec[:, :cw], in_=dnf[:, :cw])
            nc.vector.tensor_mul(out=rec[:, :cw], in0=rec[:, :cw],
                                 in1=maskbc_sb[:, csl])
            for h in range(2):
                nc.vector.tensor_mul(out=rT_sb[h][:, csl], in0=ps_rt[h][:, :cw],
                                     in1=rec[:, :cw])

            # (5) tT = Wt^T @ rT + BN partial stats
            for h in range(2):
                hsl = slice(128 * h, 128 * (h + 1))
                ps_t = psA.tile([128, 512], f32, tag="a")
                nc.tensor.matmul(ps_t[:, :cw], lhsT=wt_sb[0][:, hsl],
                                 rhs=rT_sb[0][:, csl], start=True, stop=False)
                nc.tensor.matmul(ps_t[:, :cw], lhsT=wt_sb[1][:, hsl],
                                 rhs=rT_sb[1][:, csl], start=False, stop=True)
                nc.scalar.activation(out=tT_sb[h][:, csl], in_=ps_t[:, :cw],
                                     func=AF.Copy,
                                     accum_out=sums_t[h][:, ci:ci + 1])
                sq = work.tile([128, 512], f32, tag="sq")
                nc.vector.tensor_mul(out=sq[:, :cw], in0=tT_sb[h][:, csl],
                                     in1=tT_sb[h][:, csl])
                nc.vector.reduce_sum(out=sums_q[h][:, ci:ci + 1], in_=sq[:, :cw],
                                     axis=AX.X)

            # partial-stats collectives: overlap remaining compute + resync
            if ci == nch - 3:
                emit_stats_ag(0, 0, nch - 2)
            elif ci == nch - 2:
                emit_stats_ag(1, nch - 2, nch - 1)

        # ---------- phase E: last partial + combine global BN stats ----------
        emit_stats_ag(2, nch - 1, nch)
        ag_sb = []
        for j in range(3):
            a = const.tile([4 * NCORES, 128], f32, tag=f"ag{j}", name=f"ag{j}")
            nc.sync.dma_start(out=a, in_=cc_out[j][:, :])
            ag_sb.append(a)
        ps_g = psA.tile([128, 512], f32, tag="a", name="ps_g")
        for j in range(3):
            nc.tensor.matmul(ps_g[:, 0:4], lhsT=ag_sb[j], rhs=sel_sb,
                             start=(j == 0), stop=(j == 2))
        statsT = const.tile([128, 4], f32, tag="statsT")
        nc.vector.tensor_copy(out=statsT, in_=ps_g[:, 0:4])

        scale_h, bias_h = [], []
        inv_n = 1.0 / float(N_TOT)
        for h in range(2):
            mu = small.tile([128, 1], f32, tag="mu")
            nc.vector.tensor_scalar_mul(out=mu, in0=statsT[:, h:h + 1], scalar1=inv_n)
            musq = small.tile([128, 1], f32, tag="musq")
            nc.vector.tensor_mul(out=musq, in0=mu, in1=mu)
            msq = small.tile([128, 1], f32, tag="msq")
            nc.vector.tensor_scalar(out=msq, in0=statsT[:, 2 + h:3 + h],
                                    scalar1=inv_n, scalar2=None, op0=ALU.mult)
            varp = small.tile([128, 1], f32, tag="varp")
            nc.vector.tensor_sub(out=varp, in0=msq, in1=musq)
            nc.vector.tensor_scalar_add(out=varp, in0=varp, scalar1=EPS)
            sd = small.tile([128, 1], f32, tag="sd")
            nc.scalar.activation(out=sd, in_=varp, func=AF.Sqrt)
            rsig = small.tile([128, 1], f32, tag="rsig")
            nc.vector.reciprocal(out=rsig, in_=sd)
            # one Newton step: rsig' = rsig * (1.5 - 0.5 * varp * rsig^2)
            t1 = small.tile([128, 1], f32, tag="t1")
            nc.vector.tensor_mul(out=t1, in0=rsig, in1=rsig)
            t2 = small.tile([128, 1], f32, tag="t2")
            nc.vector.tensor_mul(out=t2, in0=t1, in1=varp)
            nc.vector.tensor_scalar(out=t2, in0=t2, scalar1=-0.5, scalar2=1.5,
                                    op0=ALU.mult, op1=ALU.add)
            nc.vector.tensor_mul(out=rsig, in0=rsig, in1=t2)
            sc = small.tile([128, 1], f32, tag="sc")
            nc.vector.tensor_mul(out=sc, in0=rsig, in1=gamt_sb[h])
            bi = small.tile([128, 1], f32, tag="bi")
            nc.vector.tensor_mul(out=bi, in0=mu, in1=sc)
            nc.vector.tensor_sub(out=bi, in0=bett_sb[h], in1=bi)
            scale_h.append(sc)
            bias_h.append(bi)

        # ---------- phase F: BN apply + relu + residual (stays d-major) -----
        qn = (LP + 1023) // 1024
        for h in range(2):
            relu_sb = big.tile([128, LP], f32, tag=f"relu{h}", name=f"relu{h}")
            o_sb = big.tile([128, LP], f32, tag=f"o{h}", name=f"o{h}")
            for qi in range(qn):
                qsl = slice(1024 * qi, min(1024 * (qi + 1), LP))
                nc.scalar.activation(out=relu_sb[:, qsl], in_=tT_sb[h][:, qsl],
                                     func=AF.Relu, bias=bias_h[h],
                                     scale=scale_h[h])
                nc.vector.tensor_add(out=o_sb[:, qsl], in0=relu_sb[:, qsl],
                                     in1=featT_sb[h].bitcast(f32)[:, qsl])
                nc.sync.dma_start(out=out_d[128 * h:128 * (h + 1), qsl],
                                  in_=o_sb[:, qsl])

    nc.compile()
    return nc


def _get_nc(LP):
    if LP not in _NC_CACHE:
        _NC_CACHE[LP] = build_nc(LP)
    return _NC_CACHE[LP]


def kernel(**inputs):
    global LAST_RESULT
    feat = np.asarray(inputs["feat"], dtype=np.float32)
    bids = np.asarray(inputs["bids"])
    Wq = np.asarray(inputs["Wq"], dtype=np.float32)
    Wk = np.asarray(inputs["Wk"], dtype=np.float32)
    Wv = np.asarray(inputs["Wv"], dtype=np.float32)
    Wt = np.asarray(inputs["Wt"], dtype=np.float32)
    gamma = np.asarray(inputs["gamma"], dtype=np.float32)
    beta = np.asarray(inputs["beta"], dtype=np.float32)

    n, d = feat.shape
    assert d == D
    starts = np.searchsorted(bids, np.arange(NCORES)).astype(np.int64)
    ends = np.append(starts[1:], n)
    lens = (ends - starts).astype(np.int64)
    maxlen = int(lens.max())
    LP = max(LP_MIN, ((maxlen + 127) // 128) * 128)
    nc = _get_nc(LP)

    wqr = np.ascontiguousarray(np.concatenate([Wq] * 4, axis=1))
    wkr = np.ascontiguousarray(np.concatenate([Wk] * 4, axis=1))
    wv = np.ascontiguousarray(Wv)
    wt = Wt.astype(ml_dtypes.bfloat16)
    gamt = gamma.reshape(D, 1).copy()
    bett = beta.reshape(D, 1).copy()
    sel = np.zeros((4 * NCORES, 4), dtype=np.float32)
    for p in range(4 * NCORES):
        sel[p, p % 4] = 1.0

    in_maps = []
    for c in range(NCORES):
        seg = feat[starts[c]:ends[c]]
        L = seg.shape[0]
        featT = np.zeros((D, LP), dtype=np.float32)
        featT[:, :L] = seg.T
        maskf = np.zeros((LP, 1), dtype=np.float32)
        maskf[:L] = 1.0
        in_maps.append({
            "featT": featT, "maskf": maskf,
            "maskr": np.ascontiguousarray(maskf.reshape(1, LP)),
            "wqr": wqr, "wkr": wkr, "wv": wv, "wt": wt,
            "gamt": gamt, "bett": bett, "sel": sel,
        })

    trace_cores = None
    if os.environ.get("BASS_TRACE"):
        trace_cores = list(range(NCORES))
    res = bass_utils.run_bass_kernel_spmd(
        nc, in_maps, core_ids=list(range(NCORES)), trace_cores=trace_cores)
    LAST_RESULT = res

    out = np.empty((n, D), dtype=np.float32)
    for c in range(NCORES):
        out[starts[c]:ends[c]] = res.results[c]["out"].T[:lens[c]]
    return out


# revision 24
# speedup vs baseline: 1.0910x; 1.0275x over previous
"""Trainium2 Bass kernel for nn_ASVT_9500467658791 (ragged segment attention).

Pipeline (per point-cloud segment, one segment per NeuronCore, 8 cores):
  q/k/v = feat @ {Wq,Wk,Wv}  (1x1 convs)
  per-segment unscaled-softmax attention  r = softmax(q k^T) v
  t = r @ Wt ; BatchNorm over the full batch (training stats, synced across
  cores via tiny AllGathers) ; out = feat + relu(bn(t))

Layout strategy: everything d-major ("transposed") on chip.  The host
pre-transposes feat so no on-device transposes of the input are needed;
q^T/k^T are produced 4x-replicated across partition bands so the dqk=32
score matmuls can be packed 4-wide into the PE array with tile_position
(emitted back-to-back per group so the strips overlap in the array).
The q/k/score path runs in float32r (TF32-like).  Scores are computed
keys-major ([key, query]); the exp'd attention matrix streams as the
MOVING operand against stationary v-halves, accumulating r^T [d, q]
directly (no transposes anywhere).  Softmax denominators come from
mask-column matmuls (one per key tile, batched back-to-back so the
all-ones stationary stays resident), inverted with the fast DVE
reciprocal on all 128 partitions of the (identical-row) accumulator.
BN statistics are reduced along the free axis of t^T and synced in
three partial AllGathers: the first two overlap the remaining chunks'
compute and re-align core skew, so only the last (~1/5 of the data,
issued after only ~6us of post-barrier compute) sits on the tail.
The final output stays d-major (residual added from the exact f32 bits
of featT) and the host transposes it back during unsharding.
"""

import os
from contextlib import ExitStack

import numpy as np
import ml_dtypes

import concourse.bass as bass
import concourse.bacc as bacc
import concourse.tile as tile
from concourse import mybir
from concourse import bass_utils
from concourse.masks import make_identity

f32 = mybir.dt.float32
f32r = mybir.dt.float32r
bf16 = mybir.dt.bfloat16
AF = mybir.ActivationFunctionType
ALU = mybir.AluOpType
AX = mybir.AxisListType

NCORES = 8
D = 256
N_TOT = 16384
EPS = 1e-5
LP_MIN = 2176     # default segment pad (17 tiles); raised dynamically if needed

LAST_RESULT = None  # BassKernelResults of the most recent run (for test harness)
_NC_CACHE = {}


def _chunks(LP):
    out, c0 = [], 0
    while c0 < LP:
        out.append((c0, min(512, LP - c0)))
        c0 += 512
    return out


def build_nc(LP):
    NT = LP // 128
    chunks = _chunks(LP)
    nch = len(chunks)

    nc = bacc.Bacc("TRN2", target_bir_lowering=False, debug=False,
                   enable_asserts=True, num_devices=NCORES)

    featT_d = nc.dram_tensor("featT", [D, LP], f32r, kind="ExternalInput")
    maskf_d = nc.dram_tensor("maskf", [LP, 1], f32, kind="ExternalInput")
    maskr_d = nc.dram_tensor("maskr", [1, LP], f32, kind="ExternalInput")
    wqr_d = nc.dram_tensor("wqr", [D, 128], f32r, kind="ExternalInput")
    wkr_d = nc.dram_tensor("wkr", [D, 128], f32r, kind="ExternalInput")
    wv_d = nc.dram_tensor("wv", [D, D], f32r, kind="ExternalInput")
    wt_d = nc.dram_tensor("wt", [D, D], bf16, kind="ExternalInput")
    gamt_d = nc.dram_tensor("gamt", [D, 1], f32, kind="ExternalInput")
    bett_d = nc.dram_tensor("bett", [D, 1], f32, kind="ExternalInput")
    sel_d = nc.dram_tensor("sel", [4 * NCORES, 4], f32, kind="ExternalInput")
    out_d = nc.dram_tensor("out", [D, LP], f32, kind="ExternalOutput")

    # three partial-stats collectives + one warm-up
    cc_in = [nc.dram_tensor(f"cc_in{j}", [4, 128], f32, kind="Internal")
             for j in range(3)]
    cc_out = [nc.dram_tensor(f"cc_out{j}", [4 * NCORES, 128], f32,
                             kind="Internal", addr_space="Shared")
              for j in range(3)]
    ccw_in = nc.dram_tensor("ccw_in", [1, 128], f32, kind="Internal")
    ccw_out = nc.dram_tensor("ccw_out", [NCORES, 128], f32, kind="Internal",
                             addr_space="Shared")

    with tile.TileContext(nc) as tc, ExitStack() as ctx:
        const = ctx.enter_context(tc.tile_pool(name="const", bufs=1))
        big = ctx.enter_context(tc.tile_pool(name="big", bufs=1))
        vpool = ctx.enter_context(tc.tile_pool(name="vpool", bufs=1))
        epool = ctx.enter_context(tc.tile_pool(name="epool", bufs=2))
        work = ctx.enter_context(tc.tile_pool(name="work", bufs=3))
        small = ctx.enter_context(tc.tile_pool(name="small", bufs=4))
        # PSUM: 8 banks = scores 4 + rT accum 2 + denom 1 + tT/misc 1
        psS = ctx.enter_context(tc.tile_pool(name="psS", bufs=4, space="PSUM"))
        psV = ctx.enter_context(tc.tile_pool(name="psV", bufs=1, space="PSUM"))
        psD = ctx.enter_context(tc.tile_pool(name="psD", bufs=1, space="PSUM"))
        psA = ctx.enter_context(tc.tile_pool(name="psA", bufs=1, space="PSUM"))

        # ---------- input DMAs first (start as early as possible) ----------
        wqr_sb = [const.tile([128, 128], f32r, tag=f"wqr{h}", name=f"wqr{h}")
                  for h in range(2)]
        wkr_sb = [const.tile([128, 128], f32r, tag=f"wkr{h}", name=f"wkr{h}")
                  for h in range(2)]
        for h in range(2):
            sl = slice(128 * h, 128 * (h + 1))
            nc.sync.dma_start(out=wqr_sb[h], in_=wqr_d[sl, :])
            nc.sync.dma_start(out=wkr_sb[h], in_=wkr_d[sl, :])
        featT_sb = [[big.tile([128, cw], f32r, tag=f"featT{h}_{ci}",
                               name=f"featT{h}_{ci}")
                      for ci, (c0, cw) in enumerate(chunks)] for h in range(2)]
        for ci, (c0, cw) in enumerate(chunks):
            for h in range(2):
                nc.sync.dma_start(out=featT_sb[h][ci],
                                  in_=featT_d[128 * h:128 * (h + 1), c0:c0 + cw])

        maskf_sb = const.tile([128, NT], f32, tag="maskf")
        nc.sync.dma_start(out=maskf_sb,
                          in_=maskf_d.rearrange("(n p) one -> p (n one)", p=128))
        maskbc_sb = const.tile([128, LP], f32, tag="maskbc")
        _mr = maskr_d[0:1, :]
        nc.sync.dma_start(out=maskbc_sb, in_=bass.AP(
            tensor=_mr.tensor, offset=_mr.offset, ap=[[0, 128]] + list(_mr.ap[1:])))

        wv_sb = [const.tile([128, D], f32r, tag=f"wv{h}", name=f"wv{h}")
                 for h in range(2)]
        wt_sb = [const.tile([128, D], bf16, tag=f"wt{h}", name=f"wt{h}")
                 for h in range(2)]
        gamt_sb = [const.tile([128, 1], f32, tag=f"gam{h}", name=f"gam{h}")
                   for h in range(2)]
        bett_sb = [const.tile([128, 1], f32, tag=f"bet{h}", name=f"bet{h}")
                   for h in range(2)]
        for h in range(2):
            sl = slice(128 * h, 128 * (h + 1))
            nc.sync.dma_start(out=wv_sb[h], in_=wv_d[sl, :])
            nc.sync.dma_start(out=wt_sb[h], in_=wt_d[sl, :])
            nc.sync.dma_start(out=gamt_sb[h], in_=gamt_d[sl, :])
            nc.sync.dma_start(out=bett_sb[h], in_=bett_d[sl, :])
        sel_sb = const.tile([4 * NCORES, 4], f32, tag="sel")
        nc.sync.dma_start(out=sel_sb, in_=sel_d[:, :])

        maskb_sb = const.tile([128, NT], bf16, tag="maskb")
        nc.vector.tensor_copy(out=maskb_sb, in_=maskf_sb)
        ones128 = const.tile([128, 128], bf16, tag="ones128")
        nc.vector.memset(ones128, 1.0)
        msk128 = const.tile([128, 128], bf16, tag="msk128")
        _mb = maskb_sb[:, NT - 1:NT]
        nc.vector.tensor_copy(out=msk128, in_=bass.AP(
            tensor=_mb.tensor, offset=_mb.offset, ap=[list(_mb.ap[0]), [0, 128]]))
        ident = const.tile([128, 128], f32, tag="ident")
        make_identity(nc, ident)
        ident_b = const.tile([128, 128], bf16, tag="ident_b")
        nc.vector.tensor_copy(out=ident_b, in_=ident)

        # ---------- PE clock warm-up: junk matmuls during the input DMAs ----
        ps_w = psA.tile([128, 512], f32, tag="a")
        for i in range(60):
            nc.tensor.matmul(ps_w[:, 0:128], lhsT=ident_b, rhs=ident_b,
                             start=True, stop=True)
        warm_junk = const.tile([128, 1], f32, tag="warm_junk")
        nc.vector.tensor_copy(out=warm_junk, in_=ps_w[:, 0:1])

        # ---------- warm-up collective (runs on TOPSP during phase A) -------
        wz = const.tile([1, 128], f32, tag="wz")
        nc.vector.memset(wz, 0.0)
        nc.sync.dma_start(out=ccw_in[:, :], in_=wz)
        nc.gpsimd.collective_compute(
            "AllGather", ALU.bypass, replica_groups=[list(range(NCORES))],
            ins=[ccw_in[:, :]], outs=[ccw_out[:, :]])

        # ---------- phase A: projections ----------
        # qT_rep / kT_rep [128, LP]: each 32-row band holds the full
        # [32, LP] q^T / k^T (host replicated W 4x along columns).
        qT_sb = big.tile([128, LP], f32r, tag="qT")
        kT_sb = big.tile([128, LP], f32r, tag="kT")
        for ci, (c0, cw) in enumerate(chunks):
            csl = slice(c0, c0 + cw)
            for wrep, dst in ((wqr_sb, qT_sb), (wkr_sb, kT_sb)):
                ps = psS.tile([128, 512], f32, tag="s")
                nc.tensor.matmul(ps[:, :cw], lhsT=wrep[0], rhs=featT_sb[0][ci],
                                 start=True, stop=False)
                nc.tensor.matmul(ps[:, :cw], lhsT=wrep[1], rhs=featT_sb[1][ci],
                                 start=False, stop=True)
                nc.vector.tensor_copy(out=dst[:, csl], in_=ps[:, :cw])

        v_sb = []
        for i in range(NT):
            ci = (128 * i) // 512
            lo = 128 * i - 512 * ci
            ps = psS.tile([128, 512], f32, tag="s", name=f"psv{i}")
            nc.tensor.matmul(ps[:, 0:D], lhsT=featT_sb[0][ci][:, lo:lo + 128],
                             rhs=wv_sb[0], start=True, stop=False)
            nc.tensor.matmul(ps[:, 0:D], lhsT=featT_sb[1][ci][:, lo:lo + 128],
                             rhs=wv_sb[1], start=False, stop=True)
            vt = vpool.tile([128, D], bf16, tag=f"v{i}", name=f"v{i}")
            nc.vector.tensor_copy(out=vt, in_=ps[:, 0:D])
            v_sb.append(vt)

        rT_sb = [big.tile([128, LP], bf16, tag=f"rT{h}", name=f"rT{h}")
                 for h in range(2)]
        tT_sb = [big.tile([128, LP], f32, tag=f"tT{h}", name=f"tT{h}")
                 for h in range(2)]
        sums_t = [const.tile([128, nch], f32, tag=f"st{h}", name=f"st{h}")
                  for h in range(2)]
        sums_q = [const.tile([128, nch], f32, tag=f"sq{h}", name=f"sq{h}")
                  for h in range(2)]

        def emit_stats_ag(j, lo, hi):
            """Pack partial sums over chunks [lo, hi) and AllGather them."""
            stf = const.tile([128, 4], f32, tag=f"stf{j}", name=f"stf{j}")
            for h in range(2):
                nc.vector.reduce_sum(out=stf[:, h:h + 1],
                                     in_=sums_t[h][:, lo:hi], axis=AX.X)
                nc.vector.reduce_sum(out=stf[:, 2 + h:3 + h],
                                     in_=sums_q[h][:, lo:hi], axis=AX.X)
            ps_st = psD.tile([4, 128], f32, tag="d", name=f"ps_st{j}")
            nc.tensor.transpose(ps_st, stf, ident)
            stp = const.tile([4, 128], f32, tag=f"stp{j}", name=f"stp{j}")
            nc.vector.tensor_copy(out=stp, in_=ps_st)
            nc.sync.dma_start(out=cc_in[j][:, :], in_=stp)
            nc.gpsimd.collective_compute(
                "AllGather", ALU.bypass, replica_groups=[list(range(NCORES))],
                ins=[cc_in[j][:, :]], outs=[cc_out[j][:, :]])

        # ---------- phases B-D: attention + r^T + t^T, chunked over queries --
        for ci, (c0, cw) in enumerate(chunks):
            csl = slice(c0, c0 + cw)

            ps_rt = [psV.tile([128, 512], f32, tag=f"v{h}", name=f"psv{h}")
                     for h in range(2)]
            ps_d = psD.tile([128, 512], f32, tag="d")

            # (1) scores (4-band row packing) + exp + r^T accumulation,
            #     interleaved per kt-pair to keep the PE stream dense
            exp_of = {}
            pairs = [(kt, kt + 1 if kt + 1 < NT else None)
                     for kt in range(0, NT, 2)]
            for pi, (ka, kb) in enumerate(pairs):
                kts = [ka] + ([kb] if kb is not None else [])
                et = epool.tile([128, 1024], bf16, tag=f"e{pi}", name=f"e{pi}")
                for sub, kt in enumerate(kts):
                    i = kt % 4
                    pss = psS.tile([128, 512], f32, tag="s", name=f"pss{kt % 4}")
                    ksl = slice(128 * kt, 128 * (kt + 1))
                    bsl = slice(32 * i, 32 * (i + 1))
                    nc.tensor.matmul(pss[:, :cw], lhsT=kT_sb[bsl, ksl],
                                     rhs=qT_sb[bsl, csl],
                                     start=True, stop=True,
                                     tile_position=(32 * i, 0))
                    nc.scalar.activation(out=et[:, sub * cw:sub * cw + cw],
                                         in_=pss[:, :cw], func=AF.Exp)
                    exp_of[kt] = (et, sub)
                for sub, kt in enumerate(kts):
                    esl = slice(sub * cw, sub * cw + cw)
                    for h in range(2):
                        nc.tensor.matmul(ps_rt[h][:, :cw],
                                         lhsT=v_sb[kt][:, 128 * h:128 * (h + 1)],
                                         rhs=et[:, esl],
                                         start=(kt == 0), stop=(kt == NT - 1))

            # (3) denominators: mask-column matmuls, ones-stationary batched
            for kt in range(NT):
                et, sub = exp_of[kt]
                nc.tensor.matmul(ps_d[:, :cw],
                                 lhsT=(msk128 if kt == NT - 1 else ones128),
                                 rhs=et[:, sub * cw:sub * cw + cw],
                                 start=(kt == 0), stop=(kt == NT - 1))

            # (4) masked reciprocal (all rows of ps_d are identical)
            dnf = work.tile([128, 512], f32, tag="dnf")
            nc.vector.tensor_scalar_max(out=dnf[:, :cw], in0=ps_d[:, :cw],
                                        scalar1=1e-30)
            rec = work.tile([128, 512], f32, tag="recd")
            nc.vector.reciprocal_approx_fast(out=rec[:, :cw], in_=dnf[:, :cw])
            nc.vector.tensor_mul(out=rec[:, :cw], in0=rec[:, :cw],
                                 in1=maskbc_sb[:, csl])
            for h in range(2):
                nc.vector.tensor_mul(out=rT_sb[h][:, csl], in0=ps_rt[h][:, :cw],
                                     in1=rec[:, :cw])

            # (5) tT = Wt^T @ rT + BN partial stats
            for h in range(2):
                hsl = slice(128 * h, 128 * (h + 1))
                ps_t = psA.tile([128, 512], f32, tag="a")
                nc.tensor.matmul(ps_t[:, :cw], lhsT=wt_sb[0][:, hsl],
                                 rhs=rT_sb[0][:, csl], start=True, stop=False)
                nc.tensor.matmul(ps_t[:, :cw], lhsT=wt_sb[1][:, hsl],
                                 rhs=rT_sb[1][:, csl], start=False, stop=True)
                nc.scalar.activation(out=tT_sb[h][:, csl], in_=ps_t[:, :cw],
                                     func=AF.Copy,
                                     accum_out=sums_t[h][:, ci:ci + 1])
                sq = work.tile([128, 512], f32, tag="sq")
                nc.vector.tensor_mul(out=sq[:, :cw], in0=tT_sb[h][:, csl],
                                     in1=tT_sb[h][:, csl])
                nc.vector.reduce_sum(out=sums_q[h][:, ci:ci + 1], in_=sq[:, :cw],
                                     axis=AX.X)

            # partial-stats collectives: overlap remaining compute + resync
            if ci == 1:
                emit_stats_ag(0, 0, 2)
            elif ci == nch - 3 and nch >= 5:
                emit_stats_ag(1, 2, nch - 2)

        # ---------- phase E: last partial + combine global BN stats ----------
        emit_stats_ag(2, nch - 2, nch)
        ag_sb = []
        for j in range(3):
            a = const.tile([4 * NCORES, 128], f32, tag=f"ag{j}", name=f"ag{j}")
            nc.sync.dma_start(out=a, in_=cc_out[j][:, :])
            ag_sb.append(a)
        ps_g = psA.tile([128, 512], f32, tag="a", name="ps_g")
        for j in range(3):
            nc.tensor.matmul(ps_g[:, 0:4], lhsT=ag_sb[j], rhs=sel_sb,
                             start=(j == 0), stop=(j == 2))
        statsT = const.tile([128, 4], f32, tag="statsT")
        nc.vector.tensor_copy(out=statsT, in_=ps_g[:, 0:4])

        scale_h, bias_h = [], []
        inv_n = 1.0 / float(N_TOT)
        for h in range(2):
            mu = small.tile([128, 1], f32, tag="mu")
            nc.vector.tensor_scalar_mul(out=mu, in0=statsT[:, h:h + 1], scalar1=inv_n)
            musq = small.tile([128, 1], f32, tag="musq")
            nc.vector.tensor_mul(out=musq, in0=mu, in1=mu)
            msq = small.tile([128, 1], f32, tag="msq")
            nc.vector.tensor_scalar(out=msq, in0=statsT[:, 2 + h:3 + h],
                                    scalar1=inv_n, scalar2=None, op0=ALU.mult)
            varp = small.tile([128, 1], f32, tag="varp")
            nc.vector.tensor_sub(out=varp, in0=msq, in1=musq)
            nc.vector.tensor_scalar_add(out=varp, in0=varp, scalar1=EPS)
            sd = small.tile([128, 1], f32, tag="sd")
            nc.scalar.activation(out=sd, in_=varp, func=AF.Sqrt)
            rsig = small.tile([128, 1], f32, tag="rsig")
            nc.vector.reciprocal(out=rsig, in_=sd)
            # one Newton step: rsig' = rsig * (1.5 - 0.5 * varp * rsig^2)
            t1 = small.tile([128, 1], f32, tag="t1")
            nc.vector.tensor_mul(out=t1, in0=rsig, in1=rsig)
            t2 = small.tile([128, 1], f32, tag="t2")
            nc.vector.tensor_mul(out=t2, in0=t1, in1=varp)
            nc.vector.tensor_scalar(out=t2, in0=t2, scalar1=-0.5, scalar2=1.5,
                                    op0=ALU.mult, op1=ALU.add)
            nc.vector.tensor_mul(out=rsig, in0=rsig, in1=t2)
            sc = small.tile([128, 1], f32, tag="sc")
            nc.vector.tensor_mul(out=sc, in0=rsig, in1=gamt_sb[h])
            bi = small.tile([128, 1], f32, tag="bi")
            nc.vector.tensor_mul(out=bi, in0=mu, in1=sc)
            nc.vector.tensor_sub(out=bi, in0=bett_sb[h], in1=bi)
            scale_h.append(sc)
            bias_h.append(bi)

        # ---------- phase F: BN apply + relu + residual (stays d-major) -----
        for h in range(2):
            relu_sb = big.tile([128, LP], f32, tag=f"relu{h}", name=f"relu{h}")
            o_sb = big.tile([128, LP], f32, tag=f"o{h}", name=f"o{h}")
            for ci, (c0, cw) in enumerate(chunks):
                qsl = slice(c0, c0 + cw)
                nc.scalar.activation(out=relu_sb[:, qsl], in_=tT_sb[h][:, qsl],
                                     func=AF.Relu, bias=bias_h[h],
                                     scale=scale_h[h])
                nc.vector.tensor_add(out=o_sb[:, qsl], in0=relu_sb[:, qsl],
                                     in1=featT_sb[h][ci].bitcast(f32))
                nc.sync.dma_start(out=out_d[128 * h:128 * (h + 1), qsl],
                                  in_=o_sb[:, qsl])

    nc.compile()
    return nc


def _get_nc(LP):
    if LP not in _NC_CACHE:
        _NC_CACHE[LP] = build_nc(LP)
    return _NC_CACHE[LP]


def kernel(**inputs):
    global LAST_RESULT
    feat = np.asarray(inputs["feat"], dtype=np.float32)
    bids = np.asarray(inputs["bids"])
    Wq = np.asarray(inputs["Wq"], dtype=np.float32)
    Wk = np.asarray(inputs["Wk"], dtype=np.float32)
    Wv = np.asarray(inputs["Wv"], dtype=np.float32)
    Wt = np.asarray(inputs["Wt"], dtype=np.float32)
    gamma = np.asarray(inputs["gamma"], dtype=np.float32)
    beta = np.asarray(inputs["beta"], dtype=np.float32)

    n, d = feat.shape
    assert d == D
    starts = np.searchsorted(bids, np.arange(NCORES)).astype(np.int64)
    ends = np.append(starts[1:], n)
    lens = (ends - starts).astype(np.int64)
    maxlen = int(lens.max())
    LP = max(LP_MIN, ((maxlen + 127) // 128) * 128)
    nc = _get_nc(LP)

    wqr = np.ascontiguousarray(np.concatenate([Wq] * 4, axis=1))
    wkr = np.ascontiguousarray(np.concatenate([Wk] * 4, axis=1))
    wv = np.ascontiguousarray(Wv)
    wt = Wt.astype(ml_dtypes.bfloat16)
    gamt = gamma.reshape(D, 1).copy()
    bett = beta.reshape(D, 1).copy()
    sel = np.zeros((4 * NCORES, 4), dtype=np.float32)
    for p in range(4 * NCORES):
        sel[p, p % 4] = 1.0

    in_maps = []
    for c in range(NCORES):
        seg = feat[starts[c]:ends[c]]
        L = seg.shape[0]
        featT = np.zeros((D, LP), dtype=np.float32)
        featT[:, :L] = seg.T
        maskf = np.zeros((LP, 1), dtype=np.float32)
        maskf[:L] = 1.0
        in_maps.append({
            "featT": featT, "maskf": maskf,
            "maskr": np.ascontiguousarray(maskf.reshape(1, LP)),
            "wqr": wqr, "wkr": wkr, "wv": wv, "wt": wt,
            "gamt": gamt, "bett": bett, "sel": sel,
        })

    trace_cores = None
    if os.environ.get("BASS_TRACE"):
        trace_cores = list(range(NCORES))
    res = bass_utils.run_bass_kernel_spmd(
        nc, in_maps, core_ids=list(range(NCORES)), trace_cores=trace_cores)
    LAST_RESULT = res

    out = np.empty((n, D), dtype=np.float32)
    for c in range(NCORES):
        out[starts[c]:ends[c]] = res.results[c]["out"].T[:lens[c]]
    return out
